# revision 2
# baseline (speedup 1.0000x reference)
"""Trainium2 Bass kernel for nn_AirResistance.

out[b, t] = x[b, 0] * r**t,  r = 1 + (0.99 - 1.0) * delta_t,  out: (B, steps, 1) f32

Rank-1 structure: out = x ⊗ rpow. The power vector rpow is precomputed on the
host (tiny), broadcast to all 128 SBUF partitions, and each 128-row output tile
is produced with one per-partition-scalar multiply on the vector engine, then
streamed to HBM as a contiguous 2 MiB DMA. Batch dim B is sharded across the
8 NeuronCores (pure data parallelism, no communication).

Raw Bass (manual semaphores): this toolchain's walrus enforces at most one
sync-wait command per instruction, so waits are standalone wait_ge
instructions and every producer increments exactly one semaphore.
"""

import numpy as np

import concourse.bass as bass
from concourse import mybir
from concourse.bass_utils import run_bass_kernel_spmd

N_CORES = 8
B = 32768
STEPS = 4096
P = 128
ROWS_PER_CORE = B // N_CORES          # 4096
TILES_PER_CORE = ROWS_PER_CORE // P   # 32
K = 8                                 # output SBUF slots (double-buffer depth)

_nc_cache = None


def _build_bass():
    f32 = mybir.dt.float32
    nc = bass.Bass("TRN2", target_bir_lowering=False, debug=False)

    xt_d = nc.dram_tensor("xt", [P, TILES_PER_CORE], f32, kind="ExternalInput").ap()
    rp_d = nc.dram_tensor("rp", [P, STEPS], f32, kind="ExternalInput").ap()
    out_d = nc.dram_tensor(
        "out", [ROWS_PER_CORE, STEPS], f32, kind="ExternalOutput"
    ).ap()
    out_t = out_d.rearrange("(n p) t -> n p t", p=P)  # (32, 128, 4096)

    rp_sb = nc.alloc_sbuf_tensor("rp_sb", [P, STEPS], f32).ap()
    xt_sb = nc.alloc_sbuf_tensor("xt_sb", [P, TILES_PER_CORE], f32).ap()
    ot_sb = nc.alloc_sbuf_tensor("ot_sb", [P, K, STEPS], f32).ap()

    with (
        nc.Block() as block,
        nc.semaphore("sem_in") as sem_in,
        nc.semaphore("sem_cmp") as sem_cmp,
        nc.semaphore("sem_out") as sem_out,
    ):

        @block.sync
        def _(sync):
            sync.dma_start(out=rp_sb, in_=rp_d).then_inc(sem_in, 16)
            sync.dma_start(out=xt_sb, in_=xt_d).then_inc(sem_in, 16)
            for i in range(TILES_PER_CORE):
                sync.wait_ge(sem_cmp, i + 1)
                sync.dma_start(out=out_t[i], in_=ot_sb[:, i % K, :]).then_inc(
                    sem_out, 16
                )
            sync.wait_ge(sem_out, 16 * TILES_PER_CORE)

        @block.vector
        def _(vector):
            vector.wait_ge(sem_in, 32)
            for i in range(TILES_PER_CORE):
                if i >= K:
                    # slot i%K was last drained by output DMA #(i-K)
                    vector.wait_ge(sem_out, 16 * (i - K + 1))
                vector.tensor_scalar_mul(
                    ot_sb[:, i % K, :], rp_sb, xt_sb[:, i : i + 1]
                ).then_inc(sem_cmp, 1)

    return nc


def _get_nc():
    global _nc_cache
    if _nc_cache is None:
        _nc_cache = _build_bass()
    return _nc_cache


def kernel(steps, x, delta_t):
    steps = int(steps)
    x = np.asarray(x, dtype=np.float32)
    assert steps == STEPS and x.shape == (B, 1), (steps, x.shape)

    r32 = np.float32(1.0 + (0.99 - 1.0) * float(delta_t))
    rpow = (np.float64(r32) ** np.arange(STEPS, dtype=np.float64)).astype(np.float32)
    rp_b = np.ascontiguousarray(np.broadcast_to(rpow, (P, STEPS)))

    in_maps = []
    for c in range(N_CORES):
        xs = x[c * ROWS_PER_CORE : (c + 1) * ROWS_PER_CORE, 0]
        # xt[p, i] = x_shard[i*128 + p]: partition p of tile i holds row i*128+p
        xt = np.ascontiguousarray(xs.reshape(TILES_PER_CORE, P).T)
        in_maps.append({"xt": xt, "rp": rp_b})

    res = run_bass_kernel_spmd(_get_nc(), in_maps, list(range(N_CORES)))
    out = np.concatenate([res.results[c]["out"] for c in range(N_CORES)], axis=0)
    return out.reshape(B, STEPS, 1)


# revision 6
# speedup vs baseline: 1.1745x; 1.1745x over previous
"""Trainium2 Bass kernel for nn_AirResistance.

out[b, t] = x[b, 0] * r**t,  r = 1 + (0.99 - 1.0) * delta_t,  out: (B, steps, 1) f32

Rank-1 structure: out = x ⊗ rpow. The power vector rpow is precomputed on the
host (tiny) and broadcast to all 128 SBUF partitions; output values are
produced with per-partition-scalar multiplies on the vector engine and
streamed to HBM. Batch dim B is sharded across the 8 NeuronCores (pure data
parallelism, no communication).

Raw Bass (manual semaphores): this toolchain's walrus enforces at most one
sync-wait command per instruction, so waits are standalone wait_ge
instructions and every producer increments exactly one semaphore. Slot reuse
is gated by per-slot semaphores (a single shared completion counter would
race: DMA completions interleave per-engine across transfers).

DMA layout: HWDGE fans one descriptor per SBUF partition across the 16 SDMA
engines, and engine 15 has a fixed per-descriptor handicap (~80ns) that makes
it the kernel straggler with 16KB descriptors. Grouping 512 output rows so
partition p holds rows 4p..4p+3 (contiguous 64KB in DRAM and SBUF) gives 64KB
descriptors, amortizing the handicap 4x while keeping all 16 engines loaded.
"""

import numpy as np

import concourse.bass as bass
from concourse import mybir
from concourse.bass_utils import run_bass_kernel_spmd

N_CORES = 8
B = 32768
STEPS = 4096
P = 128
ROWS_PER_CORE = B // N_CORES          # 4096
RPP = 4                               # rows per partition per group
GROUP_ROWS = P * RPP                  # 512
N_GROUPS = ROWS_PER_CORE // GROUP_ROWS  # 8
K = 2                                 # SBUF slots (64KB/partition each)

_nc_cache = None


def _build_bass():
    f32 = mybir.dt.float32
    nc = bass.Bass("TRN2", target_bir_lowering=False, debug=False)

    xt_d = nc.dram_tensor("xt", [P, N_GROUPS, RPP], f32, kind="ExternalInput").ap()
    rp_d = nc.dram_tensor("rp", [P, STEPS], f32, kind="ExternalInput").ap()
    out_d = nc.dram_tensor(
        "out", [ROWS_PER_CORE, STEPS], f32, kind="ExternalOutput"
    ).ap()
    # group g, partition p, (j t): row 512g + 4p + j
    out_g = out_d.rearrange("(g p j) t -> g p (j t)", p=P, j=RPP)

    rp_sb = nc.alloc_sbuf_tensor("rp_sb", [P, STEPS], f32).ap()
    xt_sb = nc.alloc_sbuf_tensor("xt_sb", [P, N_GROUPS, RPP], f32).ap()
    ot_sb = nc.alloc_sbuf_tensor("ot_sb", [P, K, RPP, STEPS], f32).ap()

    with (
        nc.Block() as block,
        nc.semaphore("sem_in") as sem_in,
        nc.semaphore("sem_cmp") as sem_cmp,
        nc.semaphore("sem_s0") as sem_s0,
        nc.semaphore("sem_s1") as sem_s1,
    ):
        slot_sems = [sem_s0, sem_s1]

        @block.sync
        def _(sync):
            sync.dma_start(out=xt_sb, in_=xt_d).then_inc(sem_in, 16)
            sync.dma_start(out=rp_sb, in_=rp_d).then_inc(sem_in, 16)
            for g in range(N_GROUPS):
                # all RPP compute ops of group g done
                sync.wait_ge(sem_cmp, RPP * (g + 1))
                sync.dma_start(
                    out=out_g[g], in_=ot_sb[:, g % K, :, :]
                ).then_inc(slot_sems[g % K], 16)
            sync.wait_ge(sem_s0, 16 * ((N_GROUPS + 1) // K))
            sync.wait_ge(sem_s1, 16 * (N_GROUPS // K))

        @block.vector
        def _(vector):
            vector.wait_ge(sem_in, 32)
            for g in range(N_GROUPS):
                if g >= K:
                    # slot g%K was last drained by the group-(g-K) DMA
                    vector.wait_ge(slot_sems[g % K], 16 * ((g - K) // K + 1))
                for j in range(RPP):
                    vector.tensor_scalar_mul(
                        ot_sb[:, g % K, j, :], rp_sb, xt_sb[:, g, j : j + 1]
                    ).then_inc(sem_cmp, 1)

    return nc


def _get_nc():
    global _nc_cache
    if _nc_cache is None:
        _nc_cache = _build_bass()
    return _nc_cache


def make_in_maps(x, delta_t):
    x = np.asarray(x, dtype=np.float32)
    r32 = np.float32(1.0 + (0.99 - 1.0) * float(delta_t))
    rpow = (np.float64(r32) ** np.arange(STEPS, dtype=np.float64)).astype(np.float32)
    rp_b = np.ascontiguousarray(np.broadcast_to(rpow, (P, STEPS)))

    in_maps = []
    for c in range(N_CORES):
        xs = x[c * ROWS_PER_CORE : (c + 1) * ROWS_PER_CORE, 0]
        # xt[p, g, j] = x_shard[512g + 4p + j]
        xt = np.ascontiguousarray(
            xs.reshape(N_GROUPS, P, RPP).transpose(1, 0, 2)
        )
        in_maps.append({"xt": xt, "rp": rp_b})
    return in_maps


def kernel(steps, x, delta_t):
    steps = int(steps)
    x = np.asarray(x, dtype=np.float32)
    assert steps == STEPS and x.shape == (B, 1), (steps, x.shape)

    res = run_bass_kernel_spmd(
        _get_nc(), make_in_maps(x, delta_t), list(range(N_CORES))
    )
    out = np.concatenate([res.results[c]["out"] for c in range(N_CORES)], axis=0)
    return out.reshape(B, STEPS, 1)


# revision 7
# speedup vs baseline: 1.2342x; 1.0508x over previous
"""Trainium2 Bass kernel for nn_AirResistance.

out[b, t] = x[b, 0] * r**t,  r = 1 + (0.99 - 1.0) * delta_t,  out: (B, steps, 1) f32

Rank-1 structure: out = x ⊗ rpow. The power vector rpow is precomputed on the
host (tiny) and broadcast to all 128 SBUF partitions; output values are
produced with per-partition-scalar multiplies on the vector engine and
streamed to HBM. Batch dim B is sharded across the 8 NeuronCores (pure data
parallelism, no communication).

Raw Bass (manual semaphores): this toolchain's walrus enforces at most one
sync-wait command per instruction, so waits are standalone wait_ge
instructions and every producer increments exactly one semaphore. Slot reuse
is gated by per-slot semaphores (a single shared completion counter would
race: DMA completions interleave per-engine across transfers).

DMA layout: HWDGE fans one descriptor per SBUF partition across the 16 SDMA
engines, and engine 15 has a fixed per-descriptor handicap that makes it the
kernel straggler with 16KB descriptors. Steady-state groups cover 512 output
rows with partition p holding rows 4p..4p+3 (contiguous 64KB in DRAM and
SBUF), giving 64KB descriptors that amortize the handicap while keeping all
16 engines loaded at line rate.

Ramp: the rp table loads as two column-half DMAs, and the first groups are
small (128/128/256 rows, with group 0 stored as two column-half DMAs), so the
first output DMA issues right after the first rp half lands instead of after
a full-table load plus a full 512-row group compute.
"""

import numpy as np

import concourse.bass as bass
from concourse import mybir
from concourse.bass_utils import run_bass_kernel_spmd

N_CORES = 8
B = 32768
STEPS = 4096
HALF = STEPS // 2
P = 128
ROWS_PER_CORE = B // N_CORES          # 4096
K = 2                                 # SBUF slots (64KB/partition each)
MAX_RPP = 4

# groups: (rpp, col_split) — rows = 128*rpp; col_split only for group 0
_GROUPS = [(1, True), (1, False), (2, False)] + [(4, False)] * 7
assert sum(r for r, _ in _GROUPS) * P == ROWS_PER_CORE

_nc_cache = None


def _group_meta():
    """Per group: row0, rpp, xt_col0, list of (j-range, col-range) sub-DMAs."""
    metas = []
    row0 = 0
    col0 = 0
    for rpp, col_split in _GROUPS:
        if col_split:
            subs = [(0, rpp, 0, HALF), (0, rpp, HALF, STEPS)]
        else:
            subs = [(0, rpp, 0, STEPS)]
        metas.append({"row0": row0, "rpp": rpp, "xt_col0": col0, "subs": subs})
        row0 += P * rpp
        col0 += rpp
    return metas


def _build_bass():
    f32 = mybir.dt.float32
    nc = bass.Bass("TRN2", target_bir_lowering=False, debug=False)

    metas = _group_meta()
    n_xt_cols = sum(m["rpp"] for m in metas)

    xt_d = nc.dram_tensor("xt", [P, n_xt_cols], f32, kind="ExternalInput").ap()
    rp_d = nc.dram_tensor("rp", [P, STEPS], f32, kind="ExternalInput").ap()
    out_d = nc.dram_tensor(
        "out", [ROWS_PER_CORE, STEPS], f32, kind="ExternalOutput"
    ).ap()

    rp_sb = nc.alloc_sbuf_tensor("rp_sb", [P, STEPS], f32).ap()
    xt_sb = nc.alloc_sbuf_tensor("xt_sb", [P, n_xt_cols], f32).ap()
    ot_sb = nc.alloc_sbuf_tensor("ot_sb", [P, K, MAX_RPP, STEPS], f32).ap()

    # out AP for group g: partition p, row row0 + rpp*p + j, cols [c0:c1]
    def out_ap(m, j0, j1, c0, c1):
        rpp = m["rpp"]
        g_rows = out_d[m["row0"] : m["row0"] + P * rpp, :]
        # (p, j, t) with row = rpp*p + j
        g3 = g_rows.rearrange("(p j) t -> p j t", j=rpp)
        return g3[:, j0:j1, c0:c1]

    # TS op counts per group (for sem_cmp thresholds)
    ts_per_group = []
    for m in metas:
        n = 0
        for j0, j1, c0, c1 in m["subs"]:
            n += j1 - j0
        ts_per_group.append(n)
    cum_ts = np.concatenate([[0], np.cumsum(ts_per_group)])

    # per-slot cumulative DMA-inc totals
    slot_cum = {0: [], 1: []}  # list of cumulative inc counts after each group
    run = {0: 0, 1: 0}
    for g, m in enumerate(metas):
        run[g % K] += 16 * len(m["subs"])
        slot_cum[g % K].append(run[g % K])
    slot_after_group = {}  # group g -> slot sem value once its DMAs complete
    run = {0: 0, 1: 0}
    for g, m in enumerate(metas):
        run[g % K] += 16 * len(m["subs"])
        slot_after_group[g] = run[g % K]

    with (
        nc.Block() as block,
        nc.semaphore("sem_xt") as sem_xt,
        nc.semaphore("sem_rlo") as sem_rlo,
        nc.semaphore("sem_rhi") as sem_rhi,
        nc.semaphore("sem_cmp") as sem_cmp,
        nc.semaphore("sem_s0") as sem_s0,
        nc.semaphore("sem_s1") as sem_s1,
    ):
        slot_sems = [sem_s0, sem_s1]

        @block.sync
        def _(sync):
            sync.dma_start(out=xt_sb, in_=xt_d).then_inc(sem_xt, 16)
            sync.dma_start(out=rp_sb[:, :HALF], in_=rp_d[:, :HALF]).then_inc(
                sem_rlo, 16
            )
            sync.dma_start(out=rp_sb[:, HALF:], in_=rp_d[:, HALF:]).then_inc(
                sem_rhi, 16
            )
            done_ts = 0
            for g, m in enumerate(metas):
                for j0, j1, c0, c1 in m["subs"]:
                    done_ts += j1 - j0
                    sync.wait_ge(sem_cmp, done_ts)
                    sync.dma_start(
                        out=out_ap(m, j0, j1, c0, c1),
                        in_=ot_sb[:, g % K, j0:j1, c0:c1],
                    ).then_inc(slot_sems[g % K], 16)
            sync.wait_ge(sem_s0, slot_after_group[len(metas) - 2])
            sync.wait_ge(sem_s1, slot_after_group[len(metas) - 1])

        @block.vector
        def _(vector):
            vector.wait_ge(sem_xt, 16)
            vector.wait_ge(sem_rlo, 16)
            waited_rhi = False
            for g, m in enumerate(metas):
                if g >= K:
                    vector.wait_ge(slot_sems[g % K], slot_after_group[g - K])
                for j0, j1, c0, c1 in m["subs"]:
                    if c1 > HALF and not waited_rhi:
                        vector.wait_ge(sem_rhi, 16)
                        waited_rhi = True
                    for j in range(j0, j1):
                        vector.tensor_scalar_mul(
                            ot_sb[:, g % K, j, c0:c1],
                            rp_sb[:, c0:c1],
                            xt_sb[:, m["xt_col0"] + j : m["xt_col0"] + j + 1],
                        ).then_inc(sem_cmp, 1)

    return nc


def _get_nc():
    global _nc_cache
    if _nc_cache is None:
        _nc_cache = _build_bass()
    return _nc_cache


def make_in_maps(x, delta_t):
    x = np.asarray(x, dtype=np.float32)
    r32 = np.float32(1.0 + (0.99 - 1.0) * float(delta_t))
    rpow = (np.float64(r32) ** np.arange(STEPS, dtype=np.float64)).astype(np.float32)
    rp_b = np.ascontiguousarray(np.broadcast_to(rpow, (P, STEPS)))

    metas = _group_meta()
    n_xt_cols = sum(m["rpp"] for m in metas)

    in_maps = []
    for c in range(N_CORES):
        xs = x[c * ROWS_PER_CORE : (c + 1) * ROWS_PER_CORE, 0]
        # xt[p, col0+j] = x_shard[row0 + rpp*p + j]
        xt = np.zeros((P, n_xt_cols), dtype=np.float32)
        for m in metas:
            rpp = m["rpp"]
            blk = xs[m["row0"] : m["row0"] + P * rpp].reshape(P, rpp)
            xt[:, m["xt_col0"] : m["xt_col0"] + rpp] = blk
        in_maps.append({"xt": xt, "rp": rp_b})
    return in_maps


def kernel(steps, x, delta_t):
    steps = int(steps)
    x = np.asarray(x, dtype=np.float32)
    assert steps == STEPS and x.shape == (B, 1), (steps, x.shape)

    res = run_bass_kernel_spmd(
        _get_nc(), make_in_maps(x, delta_t), list(range(N_CORES))
    )
    out = np.concatenate([res.results[c]["out"] for c in range(N_CORES)], axis=0)
    return out.reshape(B, STEPS, 1)
